# revision 3
# baseline (speedup 1.0000x reference)
"""Self-contained Trainium2 Bass kernel for nn_AttentionGate_Wavelet.

kernel(**inputs) takes FULL unsharded inputs (as in reference.setup_inputs())
and returns the FULL output tuple (o1, o2), each [32, 128, 64, 64] float32.

Strategy: pure data parallel over batch (4 images per core, 8 cores), params
replicated. Batch-norm mean/var computed via on-device AllReduce of per-core
partial sums. All heavy matmuls run as float32r (full PE rate, ~1.5e-4 rel
err). Scale factors from skipping the downsample /4 and DWT/IWT /2 are folded
into activation scales and pre-scaled conv weights host-side.
"""
import numpy as np
from contextlib import ExitStack

import concourse.bass as bass
import concourse.tile as tile
from concourse import bacc, mybir
from concourse.bass_utils import run_bass_kernel_spmd

F32 = mybir.dt.float32
F32R = mybir.dt.float32r
AF = mybir.ActivationFunctionType
ALU = mybir.AluOpType

N_CORES = 8
BL = 4          # images per core
C = 128


def R(ap):
    return ap.bitcast(F32R)


def _build():
    nc = bacc.Bacc("TRN2", target_bir_lowering=False, debug=False,
                   num_devices=N_CORES)

    f1_d = nc.dram_tensor("frame1", [BL, C, 4096], F32, kind="ExternalInput")
    f2_d = nc.dram_tensor("frame2", [BL, C, 4096], F32, kind="ExternalInput")
    wlinT_d = nc.dram_tensor("wlinT", [128, 128], F32, kind="ExternalInput")
    ident_d = nc.dram_tensor("ident", [128, 128], F32, kind="ExternalInput")
    oh8_d = nc.dram_tensor("oh8", [8, 1024], F32, kind="ExternalInput")
    ones1_d = nc.dram_tensor("ones1", [1, 128], F32, kind="ExternalInput")
    zc_d = nc.dram_tensor("zc", [128, 40], F32, kind="ExternalInput")
    cw_d = nc.dram_tensor("cw", [2, 27, 128, 128], F32, kind="ExternalInput")
    gspa_d = nc.dram_tensor("gspa", [128, 4], F32, kind="ExternalInput")
    gfre_d = nc.dram_tensor("gfre", [128, 16], F32, kind="ExternalInput")
    fc13_d = nc.dram_tensor("fc13T", [128, 4, 16], F32, kind="ExternalInput")
    fc24_d = nc.dram_tensor("fc24T", [16, 256], F32, kind="ExternalInput")
    fcf13_d = nc.dram_tensor("fcf13T", [128, 16, 16], F32, kind="ExternalInput")
    fcf24_d = nc.dram_tensor("fcf24T", [16, 1024], F32, kind="ExternalInput")
    b16_d = nc.dram_tensor("bias16", [16, 4], F32, kind="ExternalInput")
    b128_d = nc.dram_tensor("bias128", [128, 2], F32, kind="ExternalInput")
    bf_d = nc.dram_tensor("biasf", [128, 8], F32, kind="ExternalInput")
    bng_d = nc.dram_tensor("bng", [128, 2], F32, kind="ExternalInput")
    bnb_d = nc.dram_tensor("bnb", [128, 2], F32, kind="ExternalInput")
    o1_d = nc.dram_tensor("o1", [BL, C, 4096], F32, kind="ExternalOutput")
    o2_d = nc.dram_tensor("o2", [BL, C, 4096], F32, kind="ExternalOutput")

    with tile.TileContext(nc) as tc, ExitStack() as ctx:
        cst = ctx.enter_context(tc.tile_pool(name="cst", bufs=1))
        per = ctx.enter_context(tc.tile_pool(name="per", bufs=1))
        sb = ctx.enter_context(tc.tile_pool(name="sb", bufs=1))
        ps = ctx.enter_context(tc.tile_pool(name="ps", bufs=1, space="PSUM"))
        dram = ctx.enter_context(tc.tile_pool(name="dram", bufs=1, space="DRAM"))

        # ---- constants ----
        wlinT = cst.tile([128, 128], F32)
        ident = cst.tile([128, 128], F32)
        oh8 = cst.tile([8, 1024], F32)
        ones1 = cst.tile([1, 128], F32)
        ones33 = cst.tile([33, 128], F32)
        gspa = cst.tile([128, 4], F32)
        gfre = cst.tile([128, 16], F32)
        fc13 = cst.tile([128, 4, 16], F32)
        fc24 = cst.tile([16, 256], F32)
        fcf13 = cst.tile([128, 16, 16], F32)
        fcf24 = cst.tile([16, 1024], F32)
        b16 = cst.tile([16, 4], F32)
        b128 = cst.tile([128, 2], F32)
        bf = cst.tile([128, 8], F32)
        bng = cst.tile([128, 2], F32)
        bnb = cst.tile([128, 2], F32)
        for t, d in [(wlinT, wlinT_d), (ident, ident_d), (oh8, oh8_d),
                     (ones1, ones1_d), (gspa, gspa_d), (gfre, gfre_d)]:
            nc.sync.dma_start(R(t[:]), R(d.ap()))
        nc.sync.dma_start(R(ones33[0:1, :]), R(ones1_d.ap()))
        nc.sync.dma_start(R(ones33[32:33, :]), R(ones1_d.ap()))
        for t, d in [(fc13, fc13_d), (fc24, fc24_d), (fcf13, fcf13_d),
                     (fcf24, fcf24_d), (b16, b16_d), (b128, b128_d),
                     (bf, bf_d), (bng, bng_d), (bnb, bnb_d)]:
            nc.sync.dma_start(t[:], d.ap())
        cw1 = cst.tile([128, 27, 128], F32)
        cw2 = cst.tile([128, 27, 128], F32)
        nc.sync.dma_start(R(cw1[:]), R(cw_d.ap()[0].rearrange("k ci o -> ci k o")))
        nc.sync.dma_start(R(cw2[:]), R(cw_d.ap()[1].rearrange("k ci o -> ci k o")))

        # ---- persistent: conv pad tiles (borders zeroed once), y store, stats
        cp1 = per.tile([128, 3, 1156], F32)
        cp2 = per.tile([128, 3, 1156], F32)
        for cp in (cp1, cp2):
            for g in range(3):
                pv = cp[:, g, :].rearrange("p (r c) -> p r c", r=34)
                zs = zc_d.ap()
                nc.sync.dma_start(R(pv[:, 0, :]), R(zs[:, 0:34]))
                nc.sync.dma_start(R(pv[:, 33, :]), R(zs[:, 0:34]))
                nc.sync.dma_start(R(pv[:, 1:33, 0:1]), R(zs[:, 0:32].unsqueeze(2)))
                nc.sync.dma_start(R(pv[:, 1:33, 33:34]), R(zs[:, 0:32].unsqueeze(2)))
        ysb = per.tile([128, 2, BL, 1024], F32)
        stS = per.tile([128, 16], F32)
        stQ = per.tile([128, 16], F32)

        # ================= per-image main phase =================
        for i in range(BL):
            # ---- load + downsample (sum of 2x2, no /4) ----
            f1 = sb.tile([128, 1024], F32, tag="f1")
            f2 = sb.tile([128, 1024], F32, tag="f2")
            for (fd, f) in [(f1_d, f1), (f2_d, f2)]:
                for q in range(4):
                    fr = sb.tile([128, 1024], F32, tag="fr")
                    nc.sync.dma_start(fr[:], fd.ap()[i][:, q * 1024:(q + 1) * 1024])
                    vc = fr[:].rearrange("p (h a w b) -> p h a w b", h=8, a=2, w=32, b=2)
                    t = sb.tile([128, 8, 32, 2], F32, tag="dst")
                    nc.vector.tensor_add(t[:], vc[:, :, 0], vc[:, :, 1])
                    fv = f[:, q * 256:(q + 1) * 256].rearrange("p (h w) -> p h w", h=8)
                    nc.vector.tensor_add(R(fv), t[:, :, :, 0], t[:, :, :, 1])

            # ---- transposes f1T, f2T + G1 = W_lin @ f1 ----
            f1T = sb.tile([128, 8, 128], F32, tag="fT1")
            f2T = sb.tile([128, 8, 128], F32, tag="fT2")
            for (f, fT) in [(f1, f1T), (f2, f2T)]:
                for k in range(8):
                    pt = ps.tile([128, 512], F32, tag="misc")
                    nc.tensor.transpose(R(pt[:, 0:128]), R(f[:, k * 128:(k + 1) * 128]),
                                        R(ident[:]))
                    nc.scalar.copy(R(fT[:, k, :]), pt[:, 0:128])
            G1 = sb.tile([128, 1024], F32, tag="G1")
            for ch in range(2):
                pg = ps.tile([128, 512], F32, tag="pa")
                nc.tensor.matmul(pg[:], R(wlinT[:]), R(f1[:, ch * 512:(ch + 1) * 512]),
                                 start=True, stop=True)
                nc.scalar.copy(R(G1[:, ch * 512:(ch + 1) * 512]), pg[:])

            # ---- interleaved k-loop: AT/E blocks + Q1/Q2 accumulation ----
            # ET_k = exp(AT[k·128:,:]/16) with AT = f2-block^T @ G1 ; Q1 += f2T_k^T @ ET_k
            # E_k  = exp(A[k·128:,:]/16) with A = G1-block^T @ f2  ; Q2 += f1T_k^T @ E_k
            rsE2 = sb.tile([128, 8, 2], F32, tag="rsE2")
            rsET2 = sb.tile([128, 8, 2], F32, tag="rsET2")
            q1p = ps.tile([128, 1024], F32, tag="q1")
            q2p = ps.tile([128, 1024], F32, tag="q2")
            for k in range(8):
                ET = sb.tile([128, 1024], F32, tag="eblk")
                for ch in range(2):
                    pa = ps.tile([128, 512], F32, tag="pa")
                    nc.tensor.matmul(pa[:], R(f2[:, k * 128:(k + 1) * 128]),
                                     R(G1[:, ch * 512:(ch + 1) * 512]),
                                     start=True, stop=True)
                    nc.scalar.activation(R(ET[:, ch * 512:(ch + 1) * 512]), pa[:],
                                         AF.Exp, scale=0.0625,
                                         accum_out=rsET2[:, k, ch:ch + 1])
                for ch in range(2):
                    nc.tensor.matmul(q1p[:, ch * 512:(ch + 1) * 512], R(f2T[:, k, :]),
                                     R(ET[:, ch * 512:(ch + 1) * 512]),
                                     start=(k == 0), stop=(k == 7))
                E = sb.tile([128, 1024], F32, tag="eblk")
                for ch in range(2):
                    pa = ps.tile([128, 512], F32, tag="pa")
                    nc.tensor.matmul(pa[:], R(G1[:, k * 128:(k + 1) * 128]),
                                     R(f2[:, ch * 512:(ch + 1) * 512]),
                                     start=True, stop=True)
                    nc.scalar.activation(R(E[:, ch * 512:(ch + 1) * 512]), pa[:],
                                         AF.Exp, scale=0.0625,
                                         accum_out=rsE2[:, k, ch:ch + 1])
                for ch in range(2):
                    nc.tensor.matmul(q2p[:, ch * 512:(ch + 1) * 512], R(f1T[:, k, :]),
                                     R(E[:, ch * 512:(ch + 1) * 512]),
                                     start=(k == 0), stop=(k == 7))

            # ---- normalizers: recip of row sums, transposed to [8,128] rows ----
            # Q1 (-> f1_att) divides by rowsum(A)[n] = accum of E ; Q2 by rowsum(AT)[m]
            f1_att = sb.tile([128, 1024], F32, tag="att1")
            f2_att = sb.tile([128, 1024], F32, tag="att2")
            p12h = sb.tile([128, 4], F32, tag="p12h")
            for (rs2, qp, att, pcols) in [(rsE2, q1p, f1_att, (0, 1)),
                                          (rsET2, q2p, f2_att, (2, 3))]:
                rs = sb.tile([128, 8], F32, tag="rs")
                nc.vector.tensor_add(rs[:], rs2[:, :, 0], rs2[:, :, 1])
                rec = sb.tile([128, 8], F32, tag="rec")
                nc.vector.reciprocal(rec[:], rs[:])
                ptm = ps.tile([128, 512], F32, tag="misc")
                nc.tensor.transpose(ptm[0:8, 0:128], rec[:], ident[:])
                rT = sb.tile([8, 128], F32, tag="rT")
                nc.scalar.copy(R(rT[:]), ptm[0:8, 0:128])
                for half in range(2):
                    pb = ps.tile([128, 512], F32, tag="misc")
                    for j in range(4):
                        k = half * 4 + j
                        nc.tensor.matmul(pb[:, j * 128:(j + 1) * 128],
                                         R(oh8[:, k * 128:(k + 1) * 128]), R(rT[:]),
                                         start=True, stop=True)
                    bcs = sb.tile([128, 512], F32, tag="bcs")
                    nc.scalar.copy(bcs[:], pb[:])
                    nc.vector.scalar_tensor_tensor(
                        R(att[:, half * 512:(half + 1) * 512]),
                        qp[:, half * 512:(half + 1) * 512], 1.0, bcs[:],
                        ALU.mult, ALU.mult,
                        accum_out=p12h[:, pcols[half]:pcols[half] + 1])
            pool12 = sb.tile([128, 2], F32, tag="pool12")
            nc.vector.tensor_add(pool12[:, 0:1], p12h[:, 0:1], p12h[:, 1:2])
            nc.vector.tensor_add(pool12[:, 1:2], p12h[:, 2:3], p12h[:, 3:4])

            # ---- spatial SE gates (out_e from fc1/fc2, out_q from fc3/fc4) ----
            se_cols = sb.tile([128, 2], F32, tag="secols")  # col0=out_e col1=out_q
            for w_, (qoff, bcol) in enumerate([(0, 0), (2, 1)]):
                ph = ps.tile([128, 512], F32, tag="misc")
                for blk in range(2):
                    nc.tensor.matmul(ph[0:16, 0:1], fc13[:, qoff + blk, :],
                                     pool12[:, blk:blk + 1],
                                     start=(blk == 0), stop=(blk == 1))
                t16 = sb.tile([16, 1], F32, tag="t16")
                nc.scalar.activation(t16[:], ph[0:16, 0:1], AF.Identity,
                                     bias=b16[:, bcol:bcol + 1], scale=1.0 / 4096.0)
                t16b = sb.tile([16, 1], F32, tag="t16b")
                nc.vector.tensor_scalar_mul(t16b[:], t16[:], 0.2)
                h16 = sb.tile([16, 1], F32, tag="h16")
                nc.vector.tensor_max(h16[:], t16[:], t16b[:])
                ph2 = ps.tile([128, 512], F32, tag="misc")
                nc.tensor.matmul(ph2[:, 0:1], fc24[:, w_ * 128:(w_ + 1) * 128],
                                 h16[:], start=True, stop=True)
                nc.scalar.activation(se_cols[:, w_:w_ + 1], ph2[:, 0:1], AF.Sigmoid,
                                     bias=b128[:, bcol:bcol + 1])

            # ---- spatial gates m1/m2 + gated write into conv pads ----
            mrow = sb.tile([33, 1024], F32, tag="mrow")
            for gi in range(2):  # 0 -> m1 (gspa cols 0,1), 1 -> m2 (cols 2,3)
                gp = 32 * gi
                for ch in range(2):
                    pg = ps.tile([128, 512], F32, tag="misc")
                    nc.tensor.matmul(pg[0:1, :], R(gspa[:, 2 * gi:2 * gi + 1]),
                                     R(f1_att[:, ch * 512:(ch + 1) * 512]),
                                     start=True, stop=False)
                    nc.tensor.matmul(pg[0:1, :], R(gspa[:, 2 * gi + 1:2 * gi + 2]),
                                     R(f2_att[:, ch * 512:(ch + 1) * 512]),
                                     start=False, stop=True)
                    nc.scalar.activation(R(mrow[gp:gp + 1, ch * 512:(ch + 1) * 512]),
                                         pg[0:1, :], AF.Sigmoid, scale=0.25)
            # i1_spa = out_q * f1_att * m1 -> cp1 group0 ; i2_spa = out_e * f2_att * m2
            for (gi, att, qcol, cp) in [(0, f1_att, 1, cp1), (1, f2_att, 0, cp2)]:
                cpv = cp[:, 0, :].rearrange("p (r c) -> p r c", r=34)
                for ch in range(2):
                    pb = ps.tile([128, 512], F32, tag="misc")
                    gp = 32 * gi
                    nc.tensor.matmul(pb[:], R(ones33[gp:gp + 1, :]),
                                     R(mrow[gp:gp + 1, ch * 512:(ch + 1) * 512]),
                                     start=True, stop=True)
                    out = cpv[:, 1 + ch * 16:1 + (ch + 1) * 16, 1:33]
                    nc.vector.scalar_tensor_tensor(
                        R(out),
                        att[:, ch * 512:(ch + 1) * 512].rearrange(
                            "p (r c) -> p r c", r=16),
                        se_cols[:, qcol:qcol + 1], pb[:].rearrange(
                            "p (r c) -> p r c", r=16),
                        ALU.mult, ALU.mult)

            # ---- DWT (no /2; blocks LL,LH,HL,HH) with fused pooled accum ----
            F1 = sb.tile([128, 4, 256], F32, tag="fre1")
            F2 = sb.tile([128, 4, 256], F32, tag="fre2")
            pf = sb.tile([128, 8], F32, tag="pf")
            for fi, (att, Ff) in enumerate([(f1_att, F1), (f2_att, F2)]):
                v = att[:].rearrange("p (h a w b) -> p a b h w", h=16, a=2, w=16, b=2)
                pp = sb.tile([128, 16, 16], F32, tag="wt0")
                qq = sb.tile([128, 16, 16], F32, tag="wt1")
                rr = sb.tile([128, 16, 16], F32, tag="wt2")
                ss = sb.tile([128, 16, 16], F32, tag="wt3")
                nc.vector.tensor_add(pp[:], v[:, 0, 0], v[:, 1, 0])
                nc.vector.tensor_add(qq[:], v[:, 0, 1], v[:, 1, 1])
                nc.vector.tensor_sub(rr[:], v[:, 1, 0], v[:, 0, 0])
                nc.vector.tensor_sub(ss[:], v[:, 1, 1], v[:, 0, 1])
                for bi, (a0, a1, op) in enumerate([(pp, qq, ALU.add),
                                                   (qq, pp, ALU.subtract),
                                                   (rr, ss, ALU.add),
                                                   (ss, rr, ALU.subtract)]):
                    nc.vector.scalar_tensor_tensor(
                        R(Ff[:, bi, :].rearrange("p (h w) -> p h w", h=16)),
                        a0[:], 1.0, a1[:], ALU.mult, op,
                        accum_out=pf[:, fi * 4 + bi:fi * 4 + bi + 1])

            # ---- freq SE gates ----
            sef_cols = sb.tile([128, 8], F32, tag="sefcols")  # 0:4 out_ef, 4:8 out_qf
            for w_, (qoff, bcol, ooff) in enumerate([(0, 2, 0), (8, 3, 4)]):
                phf = ps.tile([128, 512], F32, tag="misc")
                for k in range(8):
                    nc.tensor.matmul(phf[0:16, 0:1], fcf13[:, qoff + k, :],
                                     pf[:, k:k + 1], start=(k == 0), stop=(k == 7))
                t16 = sb.tile([16, 1], F32, tag="t16")
                nc.scalar.activation(t16[:], phf[0:16, 0:1], AF.Identity,
                                     bias=b16[:, bcol:bcol + 1], scale=1.0 / 2048.0)
                t16b = sb.tile([16, 1], F32, tag="t16b")
                nc.vector.tensor_scalar_mul(t16b[:], t16[:], 0.2)
                h16 = sb.tile([16, 1], F32, tag="h16")
                nc.vector.tensor_max(h16[:], t16[:], t16b[:])
                for blk in range(4):
                    ph2 = ps.tile([128, 512], F32, tag="misc")
                    nc.tensor.matmul(ph2[:, 0:1],
                                     fcf24[:, w_ * 512 + blk * 128:
                                            w_ * 512 + (blk + 1) * 128],
                                     h16[:], start=True, stop=True)
                    nc.scalar.activation(sef_cols[:, ooff + blk:ooff + blk + 1],
                                         ph2[:, 0:1], AF.Sigmoid,
                                         bias=bf[:, ooff + blk:ooff + blk + 1])

            # ---- freq gates m1f/m2f ----
            mfrow = sb.tile([33, 256], F32, tag="mfrow")
            for gi in range(2):
                pgf = ps.tile([128, 512], F32, tag="misc")
                for k in range(8):
                    src = F1 if k < 4 else F2
                    nc.tensor.matmul(pgf[0:1, 0:256],
                                     R(gfre[:, 8 * gi + k:8 * gi + k + 1]),
                                     R(src[:, k % 4, :]),
                                     start=(k == 0), stop=(k == 7))
                nc.scalar.activation(R(mfrow[32 * gi:32 * gi + 1, :]),
                                     pgf[0:1, 0:256], AF.Sigmoid, scale=0.125)
            # gating in place: F *= out_f[c-block] * m_f[pos]
            for (gi, Ff, ooff) in [(0, F1, 4), (1, F2, 0)]:
                pbf = ps.tile([128, 512], F32, tag="misc")
                gp = 32 * gi
                nc.tensor.matmul(pbf[:, 0:256], R(ones33[gp:gp + 1, :]),
                                 R(mfrow[gp:gp + 1, :]), start=True, stop=True)
                for blk in range(4):
                    nc.vector.scalar_tensor_tensor(
                        R(Ff[:, blk, :]), Ff[:, blk, :],
                        sef_cols[:, ooff + blk:ooff + blk + 1], pbf[:, 0:256],
                        ALU.mult, ALU.mult)

            # ---- IWT (no /2) straight into conv pad group 1 ----
            for (Ff, cp) in [(F1, cp1), (F2, cp2)]:
                uu = sb.tile([128, 16, 16], F32, tag="wt0")
                vv = sb.tile([128, 16, 16], F32, tag="wt1")
                ww = sb.tile([128, 16, 16], F32, tag="wt2")
                zz = sb.tile([128, 16, 16], F32, tag="wt3")
                x1 = F1v = Ff[:, 0, :].rearrange("p (h w) -> p h w", h=16)
                x2 = Ff[:, 1, :].rearrange("p (h w) -> p h w", h=16)
                x3 = Ff[:, 2, :].rearrange("p (h w) -> p h w", h=16)
                x4 = Ff[:, 3, :].rearrange("p (h w) -> p h w", h=16)
                nc.vector.tensor_sub(uu[:], x1, x2)
                nc.vector.tensor_sub(vv[:], x3, x4)
                nc.vector.tensor_add(ww[:], x1, x2)
                nc.vector.tensor_add(zz[:], x3, x4)
                ov = cp[:, 1, :].rearrange("p (r c) -> p r c", r=34)[
                    :, 1:33, 1:33].rearrange("p (h a) (w b) -> p a b h w", a=2, b=2)
                nc.vector.scalar_tensor_tensor(R(ov[:, 0, 0]), uu[:], 1.0, vv[:],
                                               ALU.mult, ALU.subtract)
                nc.vector.scalar_tensor_tensor(R(ov[:, 1, 0]), uu[:], 1.0, vv[:],
                                               ALU.mult, ALU.add)
                nc.vector.scalar_tensor_tensor(R(ov[:, 0, 1]), ww[:], 1.0, zz[:],
                                               ALU.mult, ALU.subtract)
                nc.vector.scalar_tensor_tensor(R(ov[:, 1, 1]), ww[:], 1.0, zz[:],
                                               ALU.mult, ALU.add)

            # ---- f (downsampled frame) into conv pad group 2 ----
            for (f, cp) in [(f1, cp1), (f2, cp2)]:
                pv = cp[:, 2, :].rearrange("p (r c) -> p r c", r=34)
                nc.scalar.copy(R(pv[:, 1:33, 1:33]),
                               f[:].rearrange("p (r c) -> p r c", r=32))

            # ---- conv3x3 as 27 accumulating shifted matmuls + BN stats ----
            for j, (cp, cwt) in enumerate([(cp1, cw1), (cp2, cw2)]):
                for ch in range(2):
                    pc = ps.tile([128, 512], F32, tag="pa")
                    idx = 0
                    for g in range(3):
                        pv = cp[:, g, :].rearrange("p (r c) -> p r c", r=34)
                        for dy in range(3):
                            for dx in range(3):
                                rhs = pv[:, ch * 16 + dy:ch * 16 + dy + 16,
                                         dx:dx + 32]
                                nc.tensor.matmul(pc[:], R(cwt[:, idx, :]), R(rhs),
                                                 start=(idx == 0), stop=(idx == 26))
                                idx += 1
                    scol = ((j * BL) + i) * 2 + ch
                    nc.scalar.activation(ysb[:, j, i, ch * 512:(ch + 1) * 512],
                                         pc[:], AF.Copy,
                                         accum_out=stS[:, scol:scol + 1])
                    sqs = sb.tile([128, 512], F32, tag="sqs")
                    nc.scalar.activation(sqs[:], pc[:], AF.Square,
                                         accum_out=stQ[:, scol:scol + 1])

        # ================= BN allreduce + finalize =================
        ccin = sb.tile([128, 4], F32, tag="ccin")
        nc.vector.tensor_reduce(ccin[:, 0:2],
                                stS[:].rearrange("p (j r) -> p j r", j=2),
                                mybir.AxisListType.X, ALU.add)
        nc.vector.tensor_reduce(ccin[:, 2:4],
                                stQ[:].rearrange("p (j r) -> p j r", j=2),
                                mybir.AxisListType.X, ALU.add)
        cbi = dram.tile([128, 4], F32)
        cbo = dram.tile([128, 4], F32)
        nc.sync.dma_start(cbi[:], ccin[:])
        nc.gpsimd.collective_compute(
            "AllReduce", ALU.add, replica_groups=[list(range(N_CORES))],
            ins=[cbi[:].opt()], outs=[cbo[:].opt()])
        ccall = sb.tile([128, 4], F32, tag="ccall")
        nc.sync.dma_start(ccall[:], cbo[:])

        NTOT = float(N_CORES * BL * 1024)
        mt = sb.tile([128, 2], F32, tag="mt")
        qt = sb.tile([128, 2], F32, tag="qt")
        nc.vector.tensor_scalar_mul(mt[:], ccall[:, 0:2], 1.0 / NTOT)
        nc.vector.tensor_scalar_mul(qt[:], ccall[:, 2:4], 1.0 / NTOT)
        m2t = sb.tile([128, 2], F32, tag="m2t")
        nc.vector.tensor_mul(m2t[:], mt[:], mt[:])
        var = sb.tile([128, 2], F32, tag="var")
        nc.vector.tensor_sub(var[:], qt[:], m2t[:])
        nc.vector.tensor_scalar_add(var[:], var[:], 1e-5)
        sd = sb.tile([128, 2], F32, tag="sd")
        nc.scalar.activation(sd[:], var[:], AF.Sqrt)
        rstd = sb.tile([128, 2], F32, tag="rstd")
        nc.vector.reciprocal(rstd[:], sd[:])
        sc0 = sb.tile([128, 2], F32, tag="sc0")
        nc.vector.tensor_mul(sc0[:], bng[:], rstd[:])
        scl = sb.tile([128, 2], F32, tag="scl")
        nc.vector.tensor_scalar_mul(scl[:], sc0[:], 1.0 / 16.0)
        tb = sb.tile([128, 2], F32, tag="tb")
        nc.vector.tensor_mul(tb[:], mt[:], sc0[:])
        bia = sb.tile([128, 2], F32, tag="bia")
        nc.vector.tensor_sub(bia[:], bnb[:], tb[:])
        nc.vector.tensor_scalar_mul(bia[:], bia[:], 1.0 / 16.0)
        nscl = sb.tile([128, 2], F32, tag="nscl")
        nc.vector.tensor_scalar_mul(nscl[:], scl[:], -1.0)
        nbia = sb.tile([128, 2], F32, tag="nbia")
        nc.vector.tensor_scalar_mul(nbia[:], bia[:], -1.0)

        # lrelu(BN(y))/16 then x16 bilinear upsample (3a+b taps, replicate pad)
        for i in range(BL):
            for j, od in enumerate([o1_d, o2_d]):
                yv = ysb[:, j, i, :].rearrange("p (r c) -> p r c", r=32)
                z = sb.tile([128, 34, 32], F32, tag="z")
                nc.scalar.activation(z[:, 1:33, :], yv, AF.Relu,
                                     scale=scl[:, j:j + 1], bias=bia[:, j:j + 1])
                zn = sb.tile([128, 32, 32], F32, tag="zn")
                nc.scalar.activation(zn[:], yv, AF.Relu,
                                     scale=nscl[:, j:j + 1], bias=nbia[:, j:j + 1])
                nc.vector.scalar_tensor_tensor(z[:, 1:33, :], zn[:], -0.2,
                                               z[:, 1:33, :], ALU.mult, ALU.add)
                nc.vector.tensor_copy(z[:, 0, :], z[:, 1, :])
                nc.vector.tensor_copy(z[:, 33, :], z[:, 32, :])
                t3 = sb.tile([128, 34, 32], F32, tag="t3")
                nc.scalar.mul(t3[:], z[:], 3.0)
                ur = sb.tile([128, 64, 34], F32, tag="ur")
                urv = ur[:].rearrange("p (r a) c -> p a r c", a=2)
                nc.vector.tensor_add(urv[:, 0, :, 1:33], t3[:, 1:33, :],
                                     z[:, 0:32, :])
                nc.gpsimd.tensor_add(urv[:, 1, :, 1:33], t3[:, 1:33, :],
                                     z[:, 2:34, :])
                nc.vector.tensor_copy(ur[:, :, 0:1], ur[:, :, 1:2])
                nc.vector.tensor_copy(ur[:, :, 33:34], ur[:, :, 32:33])
                for q in range(4):
                    t3c = sb.tile([128, 16, 34], F32, tag="t3c")
                    nc.scalar.mul(t3c[:], ur[:, q * 16:(q + 1) * 16, :], 3.0)
                    oct_ = sb.tile([128, 1024], F32, tag="oct")
                    ocv = oct_[:].rearrange("p (r c a) -> p a r c", c=32, a=2)
                    nc.vector.tensor_add(ocv[:, 0], t3c[:, :, 1:33],
                                         ur[:, q * 16:(q + 1) * 16, 0:32])
                    nc.gpsimd.tensor_add(ocv[:, 1], t3c[:, :, 1:33],
                                         ur[:, q * 16:(q + 1) * 16, 2:34])
                    nc.sync.dma_start(od.ap()[i][:, q * 1024:(q + 1) * 1024],
                                      oct_[:])

    nc.compile()
    return nc


_NC_CACHE = None


def _get_nc():
    global _NC_CACHE
    if _NC_CACHE is None:
        _NC_CACHE = _build()
    return _NC_CACHE


def _prep_weights(inp):
    g = lambda k: np.ascontiguousarray(np.asarray(inp[k], dtype=np.float32))
    W = {}
    W["wlinT"] = np.ascontiguousarray(g("W_lin").T)
    W["ident"] = np.eye(128, dtype=np.float32)
    oh8 = np.zeros((8, 1024), np.float32)
    for k in range(8):
        oh8[k, k * 128:(k + 1) * 128] = 1.0
    W["oh8"] = oh8
    W["ones1"] = np.ones((1, 128), np.float32)
    W["zc"] = np.zeros((128, 40), np.float32)
    cw = np.zeros((2, 27, 128, 128), np.float32)
    scales = [0.25, 1.0 / 16.0, 0.25]
    for j, name in enumerate(["conv1_w", "conv2_w"]):
        w = g(name)  # [o, 384, 3, 3]
        for gg in range(3):
            blk = w[:, gg * 128:(gg + 1) * 128].transpose(2, 3, 1, 0) * scales[gg]
            cw[j, gg * 9:(gg + 1) * 9] = blk.reshape(9, 128, 128)
    W["cw"] = cw
    W["gspa"] = np.stack([g("gate1_w")[:128], g("gate1_w")[128:],
                          g("gate2_w")[:128], g("gate2_w")[128:]], axis=1)
    W["gfre"] = np.concatenate([g("gate1f_w").reshape(8, 128).T,
                                g("gate2f_w").reshape(8, 128).T], axis=1)
    fc13 = np.zeros((128, 4, 16), np.float32)
    fc13[:, 0:2] = g("fc1_w").T.reshape(2, 128, 16).transpose(1, 0, 2)
    fc13[:, 2:4] = g("fc3_w").T.reshape(2, 128, 16).transpose(1, 0, 2)
    W["fc13T"] = fc13
    W["fc24T"] = np.concatenate([g("fc2_w").T, g("fc4_w").T], axis=1)
    fcf13 = np.zeros((128, 16, 16), np.float32)
    fcf13[:, 0:8] = g("fc1f_w").T.reshape(8, 128, 16).transpose(1, 0, 2)
    fcf13[:, 8:16] = g("fc3f_w").T.reshape(8, 128, 16).transpose(1, 0, 2)
    W["fcf13T"] = fcf13
    W["fcf24T"] = np.concatenate([g("fc2f_w").T, g("fc4f_w").T], axis=1)
    W["bias16"] = np.stack([g("fc1_b"), g("fc3_b"), g("fc1f_b"), g("fc3f_b")],
                           axis=1)
    W["bias128"] = np.stack([g("fc2_b"), g("fc4_b")], axis=1)
    W["biasf"] = np.concatenate([g("fc2f_b").reshape(4, 128).T,
                                 g("fc4f_b").reshape(4, 128).T], axis=1)
    W["bng"] = np.stack([g("bn1_g"), g("bn2_g")], axis=1)
    W["bnb"] = np.stack([g("bn1_b"), g("bn2_b")], axis=1)
    return {k: np.ascontiguousarray(v, dtype=np.float32) for k, v in W.items()}


def run(inputs, trace=False):
    nc = _get_nc()
    W = _prep_weights(inputs)
    f1 = np.ascontiguousarray(np.asarray(inputs["frame1"], np.float32)).reshape(
        32, 128, 4096)
    f2 = np.ascontiguousarray(np.asarray(inputs["frame2"], np.float32)).reshape(
        32, 128, 4096)
    in_maps = []
    for c in range(N_CORES):
        m = dict(W)
        m["frame1"] = f1[c * BL:(c + 1) * BL]
        m["frame2"] = f2[c * BL:(c + 1) * BL]
        in_maps.append(m)
    res = run_bass_kernel_spmd(nc, in_maps, core_ids=list(range(N_CORES)),
                               trace=trace)
    o1 = np.concatenate([res.results[c]["o1"] for c in range(N_CORES)], axis=0)
    o2 = np.concatenate([res.results[c]["o2"] for c in range(N_CORES)], axis=0)
    return (o1.reshape(32, 128, 64, 64), o2.reshape(32, 128, 64, 64)), res


def kernel(**inputs):
    (o1, o2), _ = run(inputs, trace=False)
    return o1, o2
